# revision 1
# baseline (speedup 1.0000x reference)
"""CrossNonLocalBlockND (B=1, Cx=64, Ci=32, D=8, H=W=48) on 8 TRN2 cores.

Sharding: data-parallel over the 18432 query positions (2304/core). Each core
computes the pooled phi/g projections from the full context redundantly,
runs a flash-attention-style score/softmax/PV pipeline for its query slice,
then the W projection. Training-mode BatchNorm statistics are obtained with a
[64,2] AllReduce across the 8 cores; each core applies BN + residual locally
and returns its [64, 2304] output slice.
"""

from contextlib import ExitStack

import numpy as np

import concourse.bass as bass
import concourse.mybir as mybir
import concourse.tile as tile
from concourse import bacc
from concourse.bass_utils import run_bass_kernel_spmd
from concourse.masks import make_identity

N_CORES = 8
CX, CI, D, H, W = 64, 32, 8, 48, 48
NQ = D * H * W            # 18432 query positions
NQC = NQ // N_CORES       # 2304 per core
NK = D * (H // 2) * (W // 2)  # 4608 key positions (after 2x2 maxpool)
EPS = 1e-5
FP = mybir.dt.float32

KT = 128                  # k-tile (score-matmul M)
NKG = NK // (4 * KT)      # 9 groups of 4 row-tiled k-tiles
QTS = [512, 512, 512, 512, 256]   # q sub-tiles per core (sum = 2304)
NCT = NQ // 512           # 36 context/conv tiles


ROW_TILE_S = True      # 4x row-tiled score matmuls via tile_position
COL_TILE = True        # col-tiled theta/conv matmuls
SKIP_AR = False         # debug: replace AllReduce with local copy
TRUNC_QT = len(QTS)


def build():
    nc = bacc.Bacc("TRN2", target_bir_lowering=False, num_devices=N_CORES)

    # ---- DRAM I/O -------------------------------------------------------
    xs = nc.dram_tensor("xs", [CX, NQC], FP, kind="ExternalInput")
    ctx = nc.dram_tensor("ctx", [CX, NQ], FP, kind="ExternalInput")
    thw = nc.dram_tensor("thw", [CI, CX], FP, kind="ExternalInput")
    thb = nc.dram_tensor("thb", [CI], FP, kind="ExternalInput")
    phw = nc.dram_tensor("phw", [CI, CX], FP, kind="ExternalInput")
    phb = nc.dram_tensor("phb", [CI], FP, kind="ExternalInput")
    gw = nc.dram_tensor("gw", [CI, CX], FP, kind="ExternalInput")
    gb = nc.dram_tensor("gb", [CI], FP, kind="ExternalInput")
    ww = nc.dram_tensor("ww", [CX, CI], FP, kind="ExternalInput")
    wb = nc.dram_tensor("wb", [CX], FP, kind="ExternalInput")
    gam = nc.dram_tensor("gam", [CX], FP, kind="ExternalInput")
    bet = nc.dram_tensor("bet", [CX], FP, kind="ExternalInput")
    out = nc.dram_tensor("out", [CX, NQC], FP, kind="ExternalOutput")

    with tile.TileContext(nc) as tc, ExitStack() as ctxs:
        singles = ctxs.enter_context(tc.tile_pool(name="singles", bufs=1))
        ctxp = ctxs.enter_context(tc.tile_pool(name="ctxp", bufs=4))
        bigs = ctxs.enter_context(tc.tile_pool(name="bigs", bufs=1))
        ep = ctxs.enter_context(tc.tile_pool(name="ep", bufs=2))
        rzp = ctxs.enter_context(tc.tile_pool(name="rzp", bufs=2))
        smalls = ctxs.enter_context(tc.tile_pool(name="smalls", bufs=1))
        dram = ctxs.enter_context(tc.tile_pool(name="dram", bufs=1, space="DRAM"))
        ps_conv = ctxs.enter_context(tc.tile_pool(name="ps_conv", bufs=2, space="PSUM"))
        ps_s = ctxs.enter_context(tc.tile_pool(name="ps_s", bufs=1, space="PSUM"))
        ps_pv = ctxs.enter_context(tc.tile_pool(name="ps_pv", bufs=2, space="PSUM"))

        # ---- weights / constants ---------------------------------------
        # pgT_aug: [65, 64]; cols 0-31 g-weights^T, cols 32-63 phi-weights^T,
        # row 64 holds the biases (paired with the ones-row of ctx tiles).
        pgT = singles.tile([CX + 1, 2 * CI], FP)
        nc.sync.dma_start(out=pgT[0:CX, 0:CI], in_=gw[:, :].rearrange("o c -> c o"))
        nc.sync.dma_start(out=pgT[0:CX, CI : 2 * CI], in_=phw[:, :].rearrange("o c -> c o"))
        nc.sync.dma_start(out=pgT[CX : CX + 1, 0:CI], in_=gb[:].rearrange("(a c) -> a c", a=1))
        nc.sync.dma_start(out=pgT[CX : CX + 1, CI : 2 * CI], in_=phb[:].rearrange("(a c) -> a c", a=1))

        thT = singles.tile([CX + 1, CI], FP)
        nc.sync.dma_start(out=thT[0:CX, :], in_=thw[:, :].rearrange("o c -> c o"))
        nc.sync.dma_start(out=thT[CX : CX + 1, :], in_=thb[:].rearrange("(a c) -> a c", a=1))

        wT = singles.tile([CI + 1, CX], FP)
        nc.sync.dma_start(out=wT[0:CI, :], in_=ww[:, :].rearrange("o i -> i o"))
        nc.sync.dma_start(out=wT[CI : CI + 1, :], in_=wb[:].rearrange("(a c) -> a c", a=1))

        gam_sb = singles.tile([CX, 1], FP)
        bet_sb = singles.tile([CX, 1], FP)
        nc.sync.dma_start(out=gam_sb[:, :], in_=gam[:].rearrange("(c a) -> c a", a=1))
        nc.sync.dma_start(out=bet_sb[:, :], in_=bet[:].rearrange("(c a) -> c a", a=1))

        ident = singles.tile([32, 32], FP)
        make_identity(nc, ident[:, :])
        ones33 = singles.tile([1, CI + 1], FP)
        nc.gpsimd.memset(ones33[:, :], 1.0)

        # ---- theta = theta_w @ x + b, replicated to 4 partition blocks --
        x_aug = singles.tile([CX + 1, NQC], FP)
        nc.sync.dma_start(out=x_aug[0:CX, :], in_=xs[:, :])
        nc.gpsimd.memset(x_aug[CX : CX + 1, :], 1.0)

        nrep = 4 if COL_TILE else 1
        theta_rt = singles.tile([32 * nrep, NQC], FP)
        for grp, (q0, qw) in enumerate([(0, 2048), (2048, 256)]):
            thps = ps_s.tile([128, 2048], FP, tag="sgrp")
            for qt in range(qw // 512 if qw >= 512 else 1):
                w_ = min(512, qw - qt * 512)
                for c in range(nrep):
                    nc.tensor.matmul(
                        out=thps[32 * c : 32 * c + 32, qt * 512 : qt * 512 + w_],
                        lhsT=thT[:, :],
                        rhs=x_aug[:, q0 + qt * 512 : q0 + qt * 512 + w_],
                        start=True,
                        stop=True,
                        tile_position=(0, 32 * c) if COL_TILE else None,
                    )
            nc.vector.tensor_copy(
                out=theta_rt[:, q0 : q0 + qw], in_=thps[0 : 32 * nrep, 0:qw])

        # ---- phi/g conv (col-tiled, concurrent) + 2x2 maxpool ----------
        # conv psum tile: partitions 0-31 = g, partitions 32-63 = phi
        if COL_TILE:
            wscr = bigs.tile([2 * CI, NQ // 2], FP)   # w-maxed [64, 9216]
            for t in range(NCT):
                ctile = ctxp.tile([CX + 1, 512], FP)
                nc.sync.dma_start(
                    out=ctile[0:CX, :], in_=ctx[:, t * 512 : (t + 1) * 512])
                nc.gpsimd.memset(ctile[CX : CX + 1, :], 1.0)
                cps = ps_conv.tile([2 * CI, 512], FP, tag="conv")
                nc.tensor.matmul(
                    out=cps[0:CI, :], lhsT=pgT[:, 0:CI], rhs=ctile[:, :],
                    start=True, stop=True, tile_position=(0, 0),
                )
                nc.tensor.matmul(
                    out=cps[CI : 2 * CI, :], lhsT=pgT[:, CI : 2 * CI], rhs=ctile[:, :],
                    start=True, stop=True, tile_position=(0, 32),
                )
                # w-pair max: [64, 256] single-input strided reduce from psum
                nc.vector.reduce_max(
                    out=wscr[:, t * 256 : (t + 1) * 256],
                    in_=cps[:, :].rearrange("p (a two) -> p a two", two=2),
                    axis=mybir.AxisListType.X,
                )
            # h-pair max over wscr [64, (d, h=48, w2=24)] -> pooled [64, 4608]
            pooled = bigs.tile([2 * CI, NK], FP)   # rows 0-31 g, 32-63 phi
            wv = wscr[:, :].rearrange("p (dh two w) -> p dh two w", two=2, w=24)
            nc.vector.tensor_max(
                out=pooled[:, :], in0=wv[:, :, 0, :], in1=wv[:, :, 1, :])
            g_pool = pooled[0:CI, :]
            p_pool = pooled[CI : 2 * CI, :]
        else:
            wscr_g = bigs.tile([CI, NQ // 2], FP)
            wscr_p = bigs.tile([CI, NQ // 2], FP)
            for t in range(NCT):
                ctile = ctxp.tile([CX + 1, 512], FP)
                nc.sync.dma_start(
                    out=ctile[0:CX, :], in_=ctx[:, t * 512 : (t + 1) * 512])
                nc.gpsimd.memset(ctile[CX : CX + 1, :], 1.0)
                for wsc, w0 in ((wscr_g, 0), (wscr_p, CI)):
                    cps = ps_conv.tile([CI, 512], FP, tag="conv")
                    nc.tensor.matmul(
                        out=cps[:, :], lhsT=pgT[:, w0 : w0 + CI], rhs=ctile[:, :],
                        start=True, stop=True,
                    )
                    nc.vector.reduce_max(
                        out=wsc[:, t * 256 : (t + 1) * 256],
                        in_=cps[:, :].rearrange("p (a two) -> p a two", two=2),
                        axis=mybir.AxisListType.X,
                    )
            pool_g = bigs.tile([CI, NK], FP)
            pool_p = bigs.tile([CI, NK], FP)
            for wsc, pl in ((wscr_g, pool_g), (wscr_p, pool_p)):
                wv = wsc[:, :].rearrange("p (dh two w) -> p dh two w", two=2, w=24)
                nc.vector.tensor_max(
                    out=pl[:, :], in0=wv[:, :, 0, :], in1=wv[:, :, 1, :])
            g_pool = pool_g[:, :]
            p_pool = pool_p[:, :]

        # ---- phi repack to row-tiled layout [128, 1152] via DMA --------
        if ROW_TILE_S:
            phi_rt = singles.tile([128, NKG * KT], FP)
            pv_phi = p_pool.rearrange("p (kg r c) -> p kg r c", kg=NKG, r=4)
            for r in range(4):
                nc.sync.dma_start(
                    out=phi_rt[32 * r : 32 * r + 32, :].rearrange(
                        "p (kg c) -> p kg c", kg=NKG),
                    in_=pv_phi[:, :, r, :],
                )
        else:
            phin = singles.tile([32, NK], FP)
            nc.sync.dma_start(out=phin[:, :], in_=p_pool)

        # ---- gxT via PE transpose: [128, 36, 33] (col 32 = ones) -------
        gxT = singles.tile([128, 36, CI + 1], FP)
        nc.gpsimd.memset(gxT[:, :, CI : CI + 1], 1.0)
        for g in range(9):
            tps = ps_conv.tile([128, 128], FP, tag="conv")
            for j in range(4):
                kt = 4 * g + j
                nc.tensor.transpose(
                    out=tps[:, 32 * j : 32 * j + 32],
                    in_=g_pool[:, kt * KT : (kt + 1) * KT],
                    identity=ident[:, :],
                )
            nc.vector.tensor_copy(out=gxT[:, 4 * g : 4 * g + 4, 0:CI], in_=tps[:, :])

        # ---- main attention loop ---------------------------------------
        y_norm = singles.tile([CI + 1, NQC], FP)
        if TRUNC_QT < len(QTS):
            nc.vector.memset(y_norm[:, :], 1.0)
        q0 = 0
        for qi, qw in enumerate(QTS[:TRUNC_QT]):
            pv = ps_pv.tile([CI + 1, 512], FP, tag="pv")
            for kg in range(NKG):
                sps = ps_s.tile([128, 2048], FP, tag="sgrp")
                for r in range(4):
                    if ROW_TILE_S:
                        nc.tensor.matmul(
                            out=sps[:, r * 512 : r * 512 + qw],
                            lhsT=phi_rt[32 * r : 32 * r + 32, kg * KT : (kg + 1) * KT],
                            rhs=theta_rt[32 * r : 32 * r + 32, q0 : q0 + qw],
                            start=True,
                            stop=True,
                            tile_position=(32 * r, 0),
                        )
                    else:
                        kt = 4 * kg + r
                        nc.tensor.matmul(
                            out=sps[:, r * 512 : r * 512 + qw],
                            lhsT=phin[:, kt * KT : (kt + 1) * KT],
                            rhs=theta_rt[0:32, q0 : q0 + qw],
                            start=True,
                            stop=True,
                        )
                et = ep.tile([128, 2048], FP, tag="e")
                sv = sps[:, :].rearrange("p (r b) -> p r b", r=4)[:, :, 0:qw]
                evw = et[:, 0 : 4 * qw].rearrange("p (r w) -> p r w", r=4)
                nc.scalar.activation(
                    out=evw, in_=sv, func=mybir.ActivationFunctionType.Exp,
                )
                for r in range(4):
                    kt = 4 * kg + r
                    nc.tensor.matmul(
                        out=pv[:, 0:qw],
                        lhsT=gxT[:, kt, :],
                        rhs=et[:, r * qw : (r + 1) * qw],
                        start=(kg == 0 and r == 0),
                        stop=(kg == NKG - 1 and r == 3),
                    )
            # normalize: y = pv / Z  (Z = row 32 of pv); also turns row 32 -> 1
            rz = rzp.tile([1, 512], FP, tag="rz")
            nc.vector.reciprocal(out=rz[:, 0:qw], in_=pv[CI : CI + 1, 0:qw])
            zscr = dram.tile([512], FP, tag="zscr")
            nc.sync.dma_start(out=zscr[0:qw], in_=rz[0:1, 0:qw])
            rzb = rzp.tile([CI + 1, 512], FP, tag="rzb")
            nc.sync.dma_start(
                out=rzb[:, 0:qw],
                in_=zscr[0:qw].rearrange("(a w) -> a w", a=1).to_broadcast((CI + 1, qw)),
            )
            nc.vector.tensor_mul(
                out=y_norm[:, q0 : q0 + qw], in0=pv[:, 0:qw], in1=rzb[:, 0:qw]
            )
            q0 += qw

        # ---- W projection ----------------------------------------------
        w_y = bigs.tile([CX, NQC], FP)
        q0 = 0
        for qi, qw in enumerate(QTS):
            wps = ps_conv.tile([CX, 512], FP, tag="conv")
            nc.tensor.matmul(
                out=wps[:, 0:qw], lhsT=wT[:, :], rhs=y_norm[:, q0 : q0 + qw],
                start=True, stop=True,
            )
            nc.scalar.copy(out=w_y[:, q0 : q0 + qw], in_=wps[:, 0:qw])
            q0 += qw

        # ---- BN stats + AllReduce --------------------------------------
        stats = smalls.tile([CX, 2], FP)
        sq = bigs.tile([CX, NQC], FP)
        nc.vector.reduce_sum(stats[:, 0:1], w_y[:, :], axis=mybir.AxisListType.X)
        nc.vector.tensor_mul(sq[:, :], w_y[:, :], w_y[:, :])
        nc.vector.reduce_sum(stats[:, 1:2], sq[:, :], axis=mybir.AxisListType.X)
        statsg = smalls.tile([CX, 2], FP)
        if SKIP_AR:
            nc.vector.tensor_scalar_mul(statsg[:, :], stats[:, :], float(N_CORES))
        else:
            b_in = dram.tile([CX, 2], FP)
            b_out = dram.tile([CX, 2], FP)
            nc.sync.dma_start(out=b_in[:, :], in_=stats[:, :])
            nc.gpsimd.collective_compute(
                "AllReduce", mybir.AluOpType.add,
                replica_groups=[list(range(N_CORES))],
                ins=[b_in[:, :].opt()],
                outs=[b_out[:, :].opt()],
            )
            nc.sync.dma_start(out=statsg[:, :], in_=b_out[:, :])

        # A = gamma * rsqrt(var+eps); B = beta - mu*A
        mu = smalls.tile([CX, 1], FP)
        ex2 = smalls.tile([CX, 1], FP)
        var = smalls.tile([CX, 1], FP)
        a0 = smalls.tile([CX, 1], FP)
        av = smalls.tile([CX, 1], FP)
        bv = smalls.tile([CX, 1], FP)
        t0 = smalls.tile([CX, 1], FP)
        nc.vector.tensor_scalar_mul(mu[:, :], statsg[:, 0:1], 1.0 / NQ)
        nc.vector.tensor_scalar_mul(ex2[:, :], statsg[:, 1:2], 1.0 / NQ)
        nc.vector.tensor_mul(t0[:, :], mu[:, :], mu[:, :])
        nc.vector.tensor_sub(var[:, :], ex2[:, :], t0[:, :])
        nc.vector.tensor_scalar_add(var[:, :], var[:, :], EPS)
        # rsqrt(v) = exp(-0.5 * ln(v + eps))
        nc.scalar.activation(
            out=a0[:, :], in_=var[:, :], func=mybir.ActivationFunctionType.Ln,
        )
        nc.scalar.activation(
            out=a0[:, :], in_=a0[:, :], func=mybir.ActivationFunctionType.Exp,
            scale=-0.5,
        )
        nc.vector.tensor_mul(av[:, :], a0[:, :], gam_sb[:, :])
        nc.vector.tensor_mul(t0[:, :], mu[:, :], av[:, :])
        nc.vector.tensor_sub(bv[:, :], bet_sb[:, :], t0[:, :])

        # ---- BN apply + residual + store -------------------------------
        bn = bigs.tile([CX, NQC], FP)
        out_sb = bigs.tile([CX, NQC], FP)
        nc.vector.tensor_scalar(
            out=bn[:, :], in0=w_y[:, :], scalar1=av[:, :], scalar2=bv[:, :],
            op0=mybir.AluOpType.mult, op1=mybir.AluOpType.add,
        )
        nc.vector.tensor_add(out_sb[:, :], bn[:, :], x_aug[0:CX, :])
        nc.sync.dma_start(out=out[:, :], in_=out_sb[:, :])

    nc.finalize()
    return nc


_NC = None


def _get_nc():
    global _NC
    if _NC is None:
        _NC = build()
    return _NC


def kernel(x, context, theta_w, theta_b, phi_w, phi_b, g_w, g_b, W_w, W_b,
           bn_gamma, bn_beta):
    nc = _get_nc()
    xf = np.ascontiguousarray(np.asarray(x, np.float32).reshape(CX, NQ))
    cf = np.ascontiguousarray(np.asarray(context, np.float32).reshape(CX, NQ))
    common = {
        "ctx": cf,
        "thw": np.ascontiguousarray(theta_w, np.float32),
        "thb": np.ascontiguousarray(theta_b, np.float32),
        "phw": np.ascontiguousarray(phi_w, np.float32),
        "phb": np.ascontiguousarray(phi_b, np.float32),
        "gw": np.ascontiguousarray(g_w, np.float32),
        "gb": np.ascontiguousarray(g_b, np.float32),
        "ww": np.ascontiguousarray(W_w, np.float32),
        "wb": np.ascontiguousarray(W_b, np.float32),
        "gam": np.ascontiguousarray(bn_gamma, np.float32),
        "bet": np.ascontiguousarray(bn_beta, np.float32),
    }
    in_maps = [
        {"xs": np.ascontiguousarray(xf[:, c * NQC : (c + 1) * NQC]), **common}
        for c in range(N_CORES)
    ]
    res = run_bass_kernel_spmd(nc, in_maps, core_ids=list(range(N_CORES)))
    full = np.concatenate([res.results[c]["out"] for c in range(N_CORES)], axis=1)
    return full.reshape(1, CX, D, H, W).astype(np.float32)



# revision 11
# speedup vs baseline: 22.7139x; 22.7139x over previous
"""CrossNonLocalBlockND (B=1, Cx=64, Ci=32, D=8, H=W=48) on one TRN2 core.

Strategy: the whole block runs on a single NeuronCore in fp16. At ~1 GFLOP
of matmul + 85M exps this fits comfortably in one core's compute budget
(~0.7 ms), and a single-core NEFF avoids all cross-core collectives and the
per-core dispatch overhead of the tunnel, which dominates wall time at this
problem size.

Math folds (exact, done host-side):
 - phi bias: maxpool(phi_w c + phi_b) = maxpool(phi_w c) + phi_b, and a
   per-query-constant score offset is softmax-invariant -> phi_b dropped.
 - g bias: attention rows sum to 1, so g_b shifts y by a per-channel
   constant; W then maps it to a per-channel constant on W_y, which
   training-mode BatchNorm subtracts exactly -> g_b dropped.
 - W bias: per-channel constant, removed by BatchNorm -> W_b dropped.
 - theta bias changes scores per-key -> kept (added after the theta matmul).

Device layout: input "cxw" fp16 [128, 18432] holds x (rows 0-63) and
context (rows 64-127), so the theta conv (PE rows 0-63) and the phi/g conv
(PE rows 64-127) run on disjoint row-groups. Scores are built 4-way
row-tiled (4 k-chunks of 128 keys per 512-query tile), exp'd on the scalar
engine, and PV-accumulated with an extra ones-column producing the softmax
denominator Z. Normalization broadcasts 1/Z via a K=1 matmul. BatchNorm
stats are exact (single core sees all 18432 positions).
"""

import numpy as np

import concourse.bass as bass
import concourse.mybir as mybir
import concourse.tile as tile
from concourse import bacc
from concourse.bass_utils import run_bass_kernel_spmd
from concourse.masks import make_identity

CX, CI, D, H, W = 64, 32, 8, 48, 48
NQ = D * H * W                 # 18432 query positions
NK = D * (H // 2) * (W // 2)   # 4608 key positions (after 2x2 maxpool)
EPS = 1e-5
FP = mybir.dt.float32
HF = mybir.dt.float16
HF_NP = np.float16

QW = 512                 # query tile width
NQT = NQ // QW           # 36 query tiles
KT = 128                 # k-chunk (score-matmul M)
NKG = NK // (4 * KT)     # 9 groups of 4 row-tiled k-chunks
N_CORES = 1


def build():
    nc = bacc.Bacc("TRN2", target_bir_lowering=False, num_devices=1)

    cxw = nc.dram_tensor("cxw", [128, NQ], HF, kind="ExternalInput")
    wb = nc.dram_tensor("wb", [128, 256], HF, kind="ExternalInput")
    wf = nc.dram_tensor("wf", [128, 4], FP, kind="ExternalInput")
    out = nc.dram_tensor("out", [CX, NQ], HF, kind="ExternalOutput")

    with tile.TileContext(nc) as tc:
        with tc.tile_pool(name="big", bufs=1) as big, \
             tc.tile_pool(name="sm", bufs=1) as sm:
            CXS = big.tile([128, NQ], HF, tag="cxs")     # x rows 0-63, ctx rows 64-127
            TH = big.tile([128, NQ], HF, tag="th")       # theta+theta_b, replicated 4x
            WY = big.tile([CX, NQ], HF, tag="wy")        # W @ y_norm
            wscr = big.tile([CX, NQ // 2], HF, tag="wscr")
            pooled = big.tile([CX, NK], FP, tag="pooled")  # rows 0-31 g, 32-63 phi

            WBs = sm.tile([128, 256], HF, tag="wbs")
            WFs = sm.tile([128, 4], FP, tag="wfs")
            nc.sync.dma_start(out=CXS[:, :], in_=cxw[:, :])
            nc.sync.dma_start(out=WBs[:, :], in_=wb[:, :])
            nc.sync.dma_start(out=WFs[:, :], in_=wf[:, :])

            ident = sm.tile([32, 32], FP, tag="ident")
            make_identity(nc, ident[:, :])
            ones33 = sm.tile([1, CI + 1], FP, tag="ones33")
            nc.vector.memset(ones33[:, :], 1.0)

            pgT = WBs[64:128, 0:64]        # cols 0-31 g_w^T, 32-63 phi_w^T
            thT4 = WBs[0:64, 64:192]       # theta_w^T replicated 4x in columns
            wT = WBs[0:32, 192:256]        # W_w^T
            thb = WFs[:, 0:1]              # theta_b replicated 4x (fp32)
            gam = WFs[0:CX, 1:2]
            bet = WFs[0:CX, 2:3]

            # ---- theta (PE rows 0-63) + phi/g conv + pool (rows 64-127) ----
            with tc.tile_pool(name="ps_a", bufs=2, space="PSUM") as ps_a, \
                 tc.tile_pool(name="ps_b", bufs=2, space="PSUM") as ps_b:
                for t in range(NQT):
                    qc = slice(t * QW, (t + 1) * QW)
                    thps = ps_a.tile([128, QW], FP, tag="th")
                    nc.tensor.matmul(out=thps[:, :], lhsT=thT4[:, :],
                                     rhs=CXS[0:64, qc], start=True, stop=True)
                    nc.vector.tensor_scalar_add(TH[:, qc], thps[:, :], thb)
                    cps = ps_b.tile([CX, QW], FP, tag="cv")
                    nc.tensor.matmul(out=cps[:, :], lhsT=pgT[:, :],
                                     rhs=CXS[64:128, qc], start=True, stop=True)
                    nc.vector.reduce_max(
                        out=wscr[:, t * (QW // 2):(t + 1) * (QW // 2)],
                        in_=cps[:, :].rearrange("p (a two) -> p a two", two=2),
                        axis=mybir.AxisListType.X,
                    )
                wv = wscr[:, :].rearrange("p (dh two w) -> p dh two w", two=2, w=24)
                nc.vector.tensor_max(out=pooled[:, :], in0=wv[:, :, 0, :],
                                     in1=wv[:, :, 1, :])

                # phi repacked row-tiled: block r rows 32r..32r+31, group kg
                # cols hold k-chunk 4*kg+r
                phi_rt = sm.tile([128, NKG * KT], HF, tag="phi_rt")
                pv_phi = pooled[CI:2 * CI, :].rearrange(
                    "p (kg r c) -> p kg r c", kg=NKG, r=4)
                for r in range(4):
                    nc.vector.tensor_copy(
                        out=phi_rt[32 * r:32 * r + 32, :].rearrange(
                            "p (kg c) -> p kg c", kg=NKG),
                        in_=pv_phi[:, :, r, :],
                    )

                # g transposed per k-chunk: [128, 36, 33], col 32 = ones (Z row)
                gxT = sm.tile([128, NK // KT, CI + 1], HF, tag="gxt")
                nc.vector.memset(gxT[:, :, CI:CI + 1], 1.0)
                for g in range(NK // KT // 4):
                    tps = ps_a.tile([128, 128], FP, tag="tp")
                    for j in range(4):
                        kt = 4 * g + j
                        nc.tensor.transpose(
                            out=tps[:, 32 * j:32 * j + 32],
                            in_=pooled[0:CI, kt * KT:(kt + 1) * KT],
                            identity=ident[:, :],
                        )
                    nc.vector.tensor_copy(
                        out=gxT[:, 4 * g:4 * g + 4, 0:CI],
                        in_=tps[:, :].rearrange("p (j c) -> p j c", j=4),
                    )

            # ---- attention + W projection ----------------------------------
            with tc.tile_pool(name="ps_s", bufs=1, space="PSUM") as ps_s, \
                 tc.tile_pool(name="ps_pv", bufs=2, space="PSUM") as ps_pv, \
                 tc.tile_pool(name="ps_m", bufs=2, space="PSUM") as ps_m, \
                 tc.tile_pool(name="ep", bufs=2) as ep, \
                 tc.tile_pool(name="yp", bufs=2) as yp:
                for qi in range(NQT):
                    qc = slice(qi * QW, (qi + 1) * QW)
                    pv = ps_pv.tile([CI + 1, QW], FP, tag="pv")
                    for kg in range(NKG):
                        sps = ps_s.tile([128, 4 * QW], FP, tag="s")
                        for r in range(4):
                            nc.tensor.matmul(
                                out=sps[:, r * QW:(r + 1) * QW],
                                lhsT=phi_rt[32 * r:32 * r + 32,
                                            kg * KT:(kg + 1) * KT],
                                rhs=TH[32 * r:32 * r + 32, qc],
                                start=True, stop=True,
                                tile_position=(32 * r, 0),
                            )
                        et = ep.tile([128, 4 * QW], HF, tag="e")
                        nc.scalar.activation(
                            out=et[:, :], in_=sps[:, :],
                            func=mybir.ActivationFunctionType.Exp,
                        )
                        for r in range(4):
                            kt = 4 * kg + r
                            nc.tensor.matmul(
                                out=pv[:, :],
                                lhsT=gxT[:, kt, :],
                                rhs=et[:, r * QW:(r + 1) * QW],
                                start=(kg == 0 and r == 0),
                                stop=(kg == NKG - 1 and r == 3),
                            )
                    # y_norm = pv / Z; 1/Z broadcast to 33 partitions via K=1 matmul
                    rz = yp.tile([1, QW], FP, tag="rz")
                    nc.vector.reciprocal(out=rz[:, :], in_=pv[CI:CI + 1, :])
                    rzp = ps_m.tile([CI + 1, QW], FP, tag="m")
                    nc.tensor.matmul(out=rzp[:, :], lhsT=ones33[:, :],
                                     rhs=rz[:, :], start=True, stop=True)
                    rzs = yp.tile([CI + 1, QW], FP, tag="rzs")
                    nc.vector.tensor_copy(out=rzs[:, :], in_=rzp[:, :])
                    yn = yp.tile([CI + 1, QW], HF, tag="yn")
                    nc.vector.tensor_mul(yn[:, :], pv[:, :], rzs[:, :])
                    wps = ps_m.tile([CX, QW], FP, tag="m")
                    nc.tensor.matmul(out=wps[:, :], lhsT=wT[:, :],
                                     rhs=yn[0:CI, :], start=True, stop=True)
                    nc.vector.tensor_copy(out=WY[:, qc], in_=wps[:, :])

            # ---- BatchNorm (exact global stats) + residual -----------------
            s1 = sm.tile([CX, 1], FP, tag="s1")
            s2p = sm.tile([CX, 9], FP, tag="s2p")
            s2 = sm.tile([CX, 1], FP, tag="s2")
            nc.vector.reduce_sum(out=s1[:, :], in_=WY[:, :],
                                 axis=mybir.AxisListType.X)
            with tc.tile_pool(name="sqp", bufs=2) as sqp:
                CH = NQ // 9
                for t in range(9):
                    ch = slice(t * CH, (t + 1) * CH)
                    sq = sqp.tile([CX, CH], FP, tag="sq")
                    nc.vector.tensor_mul(sq[:, :], WY[:, ch], WY[:, ch])
                    nc.vector.reduce_sum(out=s2p[:, t:t + 1], in_=sq[:, :],
                                         axis=mybir.AxisListType.X)
                nc.vector.reduce_sum(out=s2[:, :], in_=s2p[:, :],
                                     axis=mybir.AxisListType.X)

                mu = sm.tile([CX, 1], FP, tag="mu")
                ex2 = sm.tile([CX, 1], FP, tag="ex2")
                var = sm.tile([CX, 1], FP, tag="var")
                a0 = sm.tile([CX, 1], FP, tag="a0")
                av = sm.tile([CX, 1], FP, tag="av")
                bv = sm.tile([CX, 1], FP, tag="bv")
                t0 = sm.tile([CX, 1], FP, tag="t0")
                nc.vector.tensor_scalar_mul(mu[:, :], s1[:, :], 1.0 / NQ)
                nc.vector.tensor_scalar_mul(ex2[:, :], s2[:, :], 1.0 / NQ)
                nc.vector.tensor_mul(t0[:, :], mu[:, :], mu[:, :])
                nc.vector.tensor_sub(var[:, :], ex2[:, :], t0[:, :])
                nc.vector.tensor_scalar_add(var[:, :], var[:, :], EPS)
                # rsqrt(v) = exp(-0.5 * ln(v))
                nc.scalar.activation(out=a0[:, :], in_=var[:, :],
                                     func=mybir.ActivationFunctionType.Ln)
                nc.scalar.activation(out=a0[:, :], in_=a0[:, :],
                                     func=mybir.ActivationFunctionType.Exp,
                                     scale=-0.5)
                nc.vector.tensor_mul(av[:, :], a0[:, :], gam[:, :])
                nc.vector.tensor_mul(t0[:, :], mu[:, :], av[:, :])
                nc.vector.tensor_sub(bv[:, :], bet[:, :], t0[:, :])

                # out = (WY * av + bv) + x, written into the dead ctx rows
                # (both tensor_tensor SBUF inputs must share base partition 0)
                for t in range(9):
                    ch = slice(t * CH, (t + 1) * CH)
                    bn = sqp.tile([CX, CH], HF, tag="bn")
                    nc.vector.tensor_scalar(
                        out=bn[:, :], in0=WY[:, ch], scalar1=av[:, :],
                        scalar2=bv[:, :], op0=mybir.AluOpType.mult,
                        op1=mybir.AluOpType.add,
                    )
                    nc.vector.tensor_add(CXS[64:128, ch], bn[:, :],
                                         CXS[0:CX, ch])
                nc.sync.dma_start(out=out[:, :], in_=CXS[64:128, :])

    nc.finalize()
    return nc


_NC = None


def _get_nc():
    global _NC
    if _NC is None:
        _NC = build()
    return _NC


def build_in_map(x, context, theta_w, theta_b, phi_w, phi_b, g_w, g_b,
                 W_w, W_b, bn_gamma, bn_beta):
    xf = np.asarray(x, np.float32).reshape(CX, NQ)
    cf = np.asarray(context, np.float32).reshape(CX, NQ)
    cxw = np.concatenate([xf, cf], axis=0).astype(HF_NP)

    wbm = np.zeros((128, 256), np.float32)
    wbm[64:128, 0:32] = np.asarray(g_w, np.float32).T
    wbm[64:128, 32:64] = np.asarray(phi_w, np.float32).T
    wbm[0:64, 64:192] = np.tile(np.asarray(theta_w, np.float32).T, (1, 4))
    wbm[0:32, 192:256] = np.asarray(W_w, np.float32).T

    wfm = np.zeros((128, 4), np.float32)
    wfm[:, 0] = np.tile(np.asarray(theta_b, np.float32), 4)
    wfm[0:CX, 1] = np.asarray(bn_gamma, np.float32)
    wfm[0:CX, 2] = np.asarray(bn_beta, np.float32)

    return {"cxw": np.ascontiguousarray(cxw),
            "wb": np.ascontiguousarray(wbm.astype(HF_NP)),
            "wf": np.ascontiguousarray(wfm)}


def kernel(x, context, theta_w, theta_b, phi_w, phi_b, g_w, g_b, W_w, W_b,
           bn_gamma, bn_beta):
    nc = _get_nc()
    in_map = build_in_map(x, context, theta_w, theta_b, phi_w, phi_b,
                          g_w, g_b, W_w, W_b, bn_gamma, bn_beta)
    res = run_bass_kernel_spmd(nc, [in_map], core_ids=[0])
    full = np.asarray(res.results[0]["out"], dtype=np.float32)
    return full.reshape(1, CX, D, H, W)
